# revision 18
# baseline (speedup 1.0000x reference)
"""Trainium2 Bass kernel for nn_AutoEncoder_53781580481200 (moe_routing).

Host/device split:
  host: atoms are globally stable-sorted by symbol (the MoE routing) and
        dealt to the 8 cores in equal per-(core,symbol) slices, so every
        core runs an identical program with minimal padding (NG_s =
        ceil(ceil(C_s/8)/128)*128 per symbol, chosen at runtime from the
        data); x is stored transposed [D, NS] in bf16. Per-(core,symbol,
        image) run-boundary tables stay host-side.
  device (per core): per-symbol 2-layer MLP + energy head, matmuls bf16.
        The hard floor is PSUM evacuation: only ACT and DVE can read
        PSUM, both at 1 elem/cycle/lane for f32 sources (ACT 1.2 GHz,
        DVE 0.96 GHz), so the two ReLU+bias stages (2*NS columns total)
        are split across them, balanced by moving a few stage-2 tiles to
        ACT. Uniform 960-column tiling gives one evacuation op per tile
        per stage (ACT op = FD+222 cy, DVE op = FD+120 cy) while fitting
        PSUM: one [128,4096] f32 tile sliced into four bank-aligned
        960-col slots (L1/L2 ping-pong) whose 64-col bank tails hold the
        e-columns. L3 writes each 128-atom energy column (lhsT = h2
        chunk, rhs = w3*slope) into the tail of the slot parity it just
        drained, so PE writes never share a PSUM bank with a concurrent
        ACT/DVE read (fatal on TRN2); filled 64-col regions are drained
        to SBUF/DRAM as they complete.
  host: per-core cumsum of the (reordered) per-atom energies; per-image
        energies = prefix diffs at run boundaries + per-symbol affine
        constants x run counts (O(B)).

The pipeline is software-pipelined over 960-atom units: emission order
skews stages (L1(U) | E1(U-1), L2(U-1) | E2(U-2), L3(U-2)) so PE never
blocks on evacuations. Constants are fused into one bf16 blob -> single
DMA; the ACT activation-table load is pre-triggered by a dummy ReLU.

build_nc(nrep=K, staggered=True) wraps the pipeline in a hardware loop
(tc.For_i with staggered reset) so K back-to-back executions can be
timed in one dispatch - this is how test.py measures HW exec time under
the ~51ms axon RPC dispatch floor.
"""

import numpy as np
import ml_dtypes

import concourse.bass as bass
import concourse.bacc as bacc
import concourse.mybir as mybir
import concourse.tile as tile
from concourse.bass_utils import run_bass_kernel_spmd

# problem constants
N, D, H, S, B = 262144, 128, 128, 4, 1024
NCORES = 8

TS = 960             # atoms per compute tile (PSUM slot = 960 data cols
                     # + 64-col bank tail reserved for e-columns)
CHUNK = 2 * TS       # atoms per x load chunk (480 KB)
EREG = 64            # e-columns per parity region (the slot bank tails)

# constant blob layout (bf16, [128, CB])
_W1_OFF = 0
_W2_OFF = 512
_W3_OFF = 1024
_B1_OFF = 1028
_B2_OFF = 1032
CB = 1036

F32 = mybir.dt.float32
I32 = mybir.dt.int32
BF16 = mybir.dt.bfloat16
AF = mybir.ActivationFunctionType
ALU = mybir.AluOpType


def plan(ngs):
    """Static schedule shared by build_nc (device emission) and the host
    (e-column ordering): 960-col units, per-unit e-chunk emissions with
    parity-matched regions, and drain events."""
    ngs = tuple(int(g) for g in ngs)
    symbase = [0]
    for g in ngs:
        assert g % 128 == 0
        symbase.append(symbase[-1] + g)
    NS = symbase[-1]
    KC = NS // 128
    units = []
    for s in range(S):
        off = 0
        while off < ngs[s]:
            sz = min(TS, ngs[s] - off)
            units.append((s, off, sz))
            off += sz
    # L3 for unit u is emitted at pipeline step u+3, concurrent with the
    # E1 read of ph1 slot (u+2)%2, so its e-region parity is (u+1)%2 to
    # keep PE writes and ACT reads on different PSUM banks. Each 64-col
    # parity region is drained in 32-col halves: a half is drained (with
    # a small delay so DVE never waits on in-flight L3 matmuls) while
    # the other half fills; reuse of a half comes ~8 units after its
    # fill, well past the drain.
    HALF = EREG // 2
    DRAIN_DELAY = 2
    # Columns past the last full half per parity ("stubs") go to the ph2
    # slot bank tails instead, so the final drains read regions the next
    # For_i iteration's early L3 writes never touch (no cross-iteration
    # WAR putting the body tail on the critical path).
    nfull = [0, 0]  # full halves per parity, counted in a dry run
    dry = [0, 0]
    drycomp = [0] * S
    for u, (s, goff, sz) in enumerate(units):
        p = (u + 1) % 2
        while (drycomp[s] + 1) * 128 <= goff + sz:
            drycomp[s] += 1
            dry[p] += 1
    nfull = [dry[0] // HALF * HALF, dry[1] // HALF * HALF]

    completed = [0] * S
    cntP = [0, 0]
    cntQ = [0, 0]
    bufP = [[], []]  # gcols currently sitting in the filling half-region
    bufQ = [[], []]
    dpos = 0
    col_perm = []
    unit_events = [[] for _ in units]
    final_events = []
    for u, (s, goff, sz) in enumerate(units):
        ev = unit_events[u]
        p = (u + 1) % 2
        covered = goff + sz
        while (completed[s] + 1) * 128 <= covered:
            j = completed[s]
            completed[s] += 1
            gcol = symbase[s] // 128 + j
            if cntP[p] < nfull[p]:
                roff = cntP[p] % EREG
                ev.append(("col", s, j, p, roff))
                bufP[p].append(gcol)
                cntP[p] += 1
                if cntP[p] % HALF == 0:
                    hs = (cntP[p] - HALF) % EREG
                    drain = ("drain", p, hs, HALF, dpos)
                    if u + DRAIN_DELAY < len(units):
                        unit_events[u + DRAIN_DELAY].append(drain)
                    else:
                        final_events.append(drain)
                    col_perm.extend(bufP[p])
                    bufP[p] = []
                    dpos += HALF
            else:
                q = u % 2
                ev.append(("scol", s, j, q, cntQ[q]))
                bufQ[q].append(gcol)
                cntQ[q] += 1
    for q in (0, 1):
        n = len(bufQ[q])
        if n:
            assert n <= EREG
            final_events.append(("sdrain", q, 0, n, dpos))
            col_perm.extend(bufQ[q])
            dpos += n
    assert dpos == KC and all(completed[s] * 128 == ngs[s] for s in range(S))
    return dict(
        ngs=ngs, symbase=symbase, NS=NS, KC=KC, units=units,
        unit_events=unit_events, final_events=final_events,
        col_perm=np.asarray(col_perm, np.int64),
    )


_LAST_NGS = None  # set by prepare_inputs; build_nc default


def build_nc(ngs=None, act_e2=None, nrep=1, unroll=1, staggered=False,
             dbg_no_l3=False, dbg_no_e2=False, dbg_l3_const_src=False,
             dbg_no_drain=False):
    if ngs is None:
        ngs = _LAST_NGS
    assert ngs is not None, "call prepare_inputs first or pass ngs"
    pl = plan(ngs)
    NS, KC = pl["NS"], pl["KC"]
    units, symbase = pl["units"], pl["symbase"]
    NU = len(units)
    if act_e2 is None:
        # stage-2 evacuations moved from DVE to ACT for engine balance
        act_e2 = frozenset((9, 19, 29))
    act_e2 = frozenset(act_e2)

    nc = bacc.Bacc()

    xst_d = nc.declare_dram_parameter("xst", [D, NS], BF16, isOutput=False)
    cst_d = nc.declare_dram_parameter("cst", [128, CB], BF16, isOutput=False)
    e_d = nc.declare_dram_parameter("e", [128, KC], F32, isOutput=True)

    with tile.TileContext(nc) as tc:
        with (
            tc.tile_pool(name="const", bufs=1) as cpool,
            tc.tile_pool(name="xload", bufs=4) as gpool,
            tc.tile_pool(name="h1", bufs=4) as h1pool,
            tc.tile_pool(name="psum", bufs=1, space="PSUM") as ppool,
        ):
            # ---- ACT table preload: dummy ReLU on a zeroed tile ----
            zt = cpool.tile([128, 1], F32, tag="zt")
            nc.vector.memset(zt[:], 0.0)
            zt2 = cpool.tile([128, 1], F32, tag="zt2")
            nc.scalar.activation(out=zt2[:], in_=zt[:], func=AF.Relu)

            # ---- preload constants: one bf16 DMA ----
            cst_sb = cpool.tile([128, CB], BF16, tag="cst")
            nc.sync.dma_start(out=cst_sb[:], in_=cst_d[:])
            w1_sb = [
                cst_sb[:, _W1_OFF + 128 * s : _W1_OFF + 128 * (s + 1)]
                for s in range(S)
            ]
            w2_sb = [
                cst_sb[:, _W2_OFF + 128 * s : _W2_OFF + 128 * (s + 1)]
                for s in range(S)
            ]
            w3_sb = [cst_sb[:, _W3_OFF + s : _W3_OFF + s + 1] for s in range(S)]
            b1f = cpool.tile([128, S], F32, tag="b1f")
            nc.vector.tensor_copy(
                out=b1f[:], in_=cst_sb[:, _B1_OFF : _B1_OFF + S]
            )
            b2f = cpool.tile([128, S], F32, tag="b2f")
            nc.vector.tensor_copy(
                out=b2f[:], in_=cst_sb[:, _B2_OFF : _B2_OFF + S]
            )
            b1_sb = [b1f[:, s : s + 1] for s in range(S)]
            b2_sb = [b2f[:, s : s + 1] for s in range(S)]

            # h2 ring: full-size so the 128-atom L3 chunks are contiguous
            # even though they straddle 960-col stage-2 tiles
            h2ring = cpool.tile([128, NS], BF16, tag="h2ring")
            e_sb = cpool.tile([128, KC], F32, tag="e_sb")

            # PSUM: one [128,4096] f32 tile = all 8 banks, sliced into
            # bank-aligned slots so concurrent PE writes and ACT/DVE reads
            # never share a bank:
            #  banks 0-1: ph1 slot0 [0:960]   + e-region parity0 [960:1024]
            #  banks 2-3: ph1 slot1 [1024:1984] + e-region parity1 [1984:2048]
            #  banks 4-5: ph2 slot0 [2048:3008]
            #  banks 6-7: ph2 slot1 [3072:4032]
            P = ppool.tile([128, 4096], F32, tag="P")
            ph1 = [P[:, 0:960], P[:, 1024:1984]]
            ereg = [P[:, 960:1024], P[:, 1984:2048]]
            ph2 = [P[:, 2048:3008], P[:, 3072:4032]]
            sreg = [P[:, 3008:3072], P[:, 4032:4096]]  # stub e-columns

            def spans(size):
                # per-slot matmul splits at the bank boundary (col 512)
                if size <= 512:
                    return [(0, size)]
                return [(0, 512), (512, size)]

            def evac(eng, out, in_, bias):
                if eng == "act":
                    nc.scalar.activation(
                        out=out, in_=in_, func=AF.Relu, bias=bias
                    )
                else:
                    nc.vector.tensor_scalar(
                        out=out, in0=in_, scalar1=bias, scalar2=0.0,
                        op0=ALU.add, op1=ALU.max,
                    )

            def body():
                xch = {}
                h1_u = {}

                def load_chunk(s, ci):
                    if (s, ci) in xch:
                        return
                    base = symbase[s] + ci * CHUNK
                    sz = min(CHUNK, ngs[s] - ci * CHUNK)
                    xt = gpool.tile([128, CHUNK], BF16, tag="xtc")
                    nc.sync.dma_start(
                        out=xt[:, :sz], in_=xst_d[:, base : base + sz]
                    )
                    xch[(s, ci)] = xt

                for T in range(NU + 3):
                    # L1 for unit T
                    if T < NU:
                        s, goff, sz = units[T]
                        ci, co = divmod(goff, CHUNK)
                        load_chunk(s, ci)
                        slot = ph1[T % 2]
                        for c0, c1 in spans(sz):
                            nc.tensor.matmul(
                                out=slot[:, c0:c1], lhsT=w1_sb[s],
                                rhs=xch[(s, ci)][:, co + c0 : co + c1],
                                start=True, stop=True,
                            )
                    # E1 + L2 for unit T-1
                    U = T - 1
                    if 0 <= U < NU:
                        s, goff, sz = units[U]
                        h1_sb = h1pool.tile([128, TS], BF16, tag="h1_sb")
                        evac("act", h1_sb[:, :sz], ph1[U % 2][:, :sz], b1_sb[s])
                        h1_u[U] = h1_sb
                        slot = ph2[U % 2]
                        for c0, c1 in spans(sz):
                            nc.tensor.matmul(
                                out=slot[:, c0:c1], lhsT=w2_sb[s],
                                rhs=h1_sb[:, c0:c1],
                                start=True, stop=True,
                            )
                    # E2 for unit T-2
                    U = T - 2
                    if 0 <= U < NU:
                        s, goff, sz = units[U]
                        gflat = symbase[s] + goff
                        h1_u.pop(U, None)
                        if not dbg_no_e2:
                            evac(
                                "act" if U in act_e2 else "dve",
                                h2ring[:, gflat : gflat + sz],
                                ph2[U % 2][:, :sz], b2_sb[s],
                            )
                    # L3 + drains for unit T-3 (one step behind E2, so the
                    # PE FIFO never blocks on a same-step DVE op)
                    U = T - 3
                    if 0 <= U < NU and not dbg_no_l3:
                        for ev in pl["unit_events"][U]:
                            if ev[0] in ("col", "scol"):
                                _, es, j, p, roff = ev
                                reg = sreg if ev[0] == "scol" else ereg
                                lhsT = (
                                    cst_sb[:, 0:128] if dbg_l3_const_src
                                    else h2ring[
                                        :, symbase[es] + 128 * j :
                                        symbase[es] + 128 * (j + 1)
                                    ]
                                )
                                nc.tensor.matmul(
                                    out=reg[p][:, roff : roff + 1],
                                    lhsT=lhsT,
                                    rhs=w3_sb[es],
                                    start=True, stop=True,
                                )
                            elif not dbg_no_drain:
                                _, p, hs, n, dpos = ev
                                reg = sreg if ev[0] == "sdrain" else ereg
                                nc.vector.tensor_copy(
                                    out=e_sb[:, dpos : dpos + n],
                                    in_=reg[p][:, hs : hs + n],
                                )
                                nc.sync.dma_start(
                                    out=e_d[:, dpos : dpos + n],
                                    in_=e_sb[:, dpos : dpos + n],
                                )
                for ev in ([] if dbg_no_l3 else pl["final_events"]):
                    _, p, hs, n, dpos = ev
                    reg = sreg if ev[0] == "sdrain" else ereg
                    nc.vector.tensor_copy(
                        out=e_sb[:, dpos : dpos + n], in_=reg[p][:, hs : hs + n]
                    )
                    nc.sync.dma_start(
                        out=e_d[:, dpos : dpos + n],
                        in_=e_sb[:, dpos : dpos + n],
                    )

            if nrep == 1:
                body()
            else:
                assert nrep % unroll == 0
                with tc.For_i(0, nrep // unroll, 1, staggered_reset=staggered):
                    for _ in range(unroll):
                        body()
    nc.finalize()
    return nc


def prepare_inputs(x, symbol_ids, image_ids, W1, b1, W2, b2, W3, b3, slope,
                   intercept):
    """Global stable sort by symbol, equal per-(core,symbol) dealing;
    run-boundary tables kept host-side. Returns (in_maps, metas)."""
    global _LAST_NGS
    x = np.ascontiguousarray(np.asarray(x, dtype=np.float32))
    sym = np.asarray(symbol_ids, dtype=np.int32)
    img = np.asarray(image_ids, dtype=np.int32)
    W1 = np.ascontiguousarray(np.asarray(W1, np.float32))
    W2 = np.ascontiguousarray(np.asarray(W2, np.float32))
    W3 = np.asarray(W3, np.float32)
    b1 = np.ascontiguousarray(np.asarray(b1, np.float32))
    b2 = np.ascontiguousarray(np.asarray(b2, np.float32))
    b3 = np.asarray(b3, np.float32)
    slope = np.asarray(slope, np.float32)
    intercept = np.asarray(intercept, np.float32)

    W3c = (W3 * slope[:, None]).astype(np.float32)
    cvec = (slope * b3 + intercept).astype(np.float32).reshape(1, S)

    cst = np.zeros((128, CB), ml_dtypes.bfloat16)
    for s in range(S):
        cst[:, _W1_OFF + 128 * s : _W1_OFF + 128 * (s + 1)] = W1[s]
        cst[:, _W2_OFF + 128 * s : _W2_OFF + 128 * (s + 1)] = W2[s]
        cst[:, _W3_OFF + s] = W3c[s]
        cst[:, _B1_OFF + s] = b1[s]
        cst[:, _B2_OFF + s] = b2[s]

    order = np.argsort(sym, kind="stable").astype(np.int64)
    counts = np.bincount(sym, minlength=S)
    starts = np.concatenate([[0], np.cumsum(counts)])
    gs = [(int(counts[s]) + NCORES - 1) // NCORES for s in range(S)]
    ngs = tuple((g + 127) // 128 * 128 for g in gs)
    _LAST_NGS = ngs
    pl = plan(ngs)
    NS, symbase = pl["NS"], pl["symbase"]

    in_maps, metas = [], []
    for k in range(NCORES):
        xs = np.zeros((NS, D), ml_dtypes.bfloat16)
        bnd = np.zeros(S * (B + 1), np.int64)
        cnts = np.zeros((S, B), np.int64)
        for s in range(S):
            lo = starts[s] + k * gs[s]
            hi = min(starts[s] + (k + 1) * gs[s], starts[s + 1])
            gidx = order[lo:hi]
            cnt = hi - lo
            base = symbase[s]
            xs[base : base + cnt] = x[gidx]
            gimg = img[gidx]
            ends = np.searchsorted(gimg, np.arange(B), "right")
            bnd[s * (B + 1) : s * (B + 1) + B] = base + ends - 1
            bnd[s * (B + 1) + B] = base + ngs[s] - 1
            cnts[s] = np.diff(np.concatenate([[0], ends]))
        xst = np.ascontiguousarray(xs.T)  # [D, NS] bf16
        in_maps.append(dict(xst=xst, cst=cst))
        metas.append((bnd, cnts, cvec))
    return in_maps, (metas, pl)


def finish_output(results, metas):
    """Per-image energies from device per-atom energies: host prefix sums +
    O(B) boundary diffs."""
    metas, pl = metas
    NS, KC = pl["NS"], pl["KC"]
    col_perm = pl["col_perm"]
    out = np.zeros(B, np.float32)
    for k in range(NCORES):
        bnd, cnts, cvec = metas[k]
        e2d = np.asarray(results[k]["e"], np.float64)  # [128, KC]
        e_flat = np.zeros((KC, 128), np.float64)
        e_flat[col_perm] = e2d.T
        gp = np.cumsum(e_flat.reshape(-1))
        q = bnd
        gpv = np.where(q >= 0, gp[np.maximum(q, 0)], 0.0)
        t = np.concatenate([[0.0], gpv])
        rs = (t[1:] - t[:-1]).reshape(S, B + 1)[:, :B]
        rs = rs + cvec.reshape(S, 1) * cnts  # per-symbol affine constants
        out += rs.sum(axis=0).astype(np.float32)
    return out


_NC_CACHE = {}


def kernel(**inputs):
    in_maps, metas = prepare_inputs(**inputs)
    ngs = metas[1]["ngs"]
    if ngs not in _NC_CACHE:
        _NC_CACHE[ngs] = build_nc(ngs)
    res = run_bass_kernel_spmd(_NC_CACHE[ngs], in_maps, list(range(NCORES)))
    return finish_output(res.results, metas)


# revision 33
# speedup vs baseline: 1.0801x; 1.0801x over previous
"""Trainium2 Bass kernel for nn_AutoEncoder_53781580481200 (moe_routing).

Host/device split:
  host: atoms are globally stable-sorted by symbol (the MoE routing) and
        dealt to the 8 cores in equal per-(core,symbol) slices, so every
        core runs an identical program with minimal padding (NG_s =
        ceil(ceil(C_s/8)/128)*128 per symbol, chosen at runtime from the
        data); x is stored transposed [D, NS] in bf16. Per-(core,symbol,
        image) run-boundary tables stay host-side.
  device (per core): per-symbol 2-layer MLP + energy head, matmuls bf16.
        The hard floor is PSUM evacuation: only ACT and DVE can read
        PSUM, both at 1 elem/cycle/lane for f32 sources (ACT 1.2 GHz,
        DVE 0.96 GHz), so the two ReLU+bias stages (2*NS columns total)
        are split across them, balanced by moving a few stage-2 tiles to
        ACT. Uniform 960-column tiling gives one evacuation op per tile
        per stage (ACT op = FD+222 cy, DVE op = FD+120 cy) while fitting
        PSUM: one [128,4096] f32 tile sliced into four bank-aligned
        960-col slots (L1/L2 ping-pong) whose 64-col bank tails hold the
        e-columns. L3 writes each 128-atom energy column (lhsT = h2
        chunk, rhs = w3*slope) into the tail of the slot parity it just
        drained, so PE writes never share a PSUM bank with a concurrent
        ACT/DVE read (fatal on TRN2); filled 64-col regions are drained
        to SBUF/DRAM as they complete.
  host: per-core cumsum of the (reordered) per-atom energies; per-image
        energies = prefix diffs at run boundaries + per-symbol affine
        constants x run counts (O(B)).

The pipeline is software-pipelined over 960-atom units: emission order
skews stages (L1(U) | E1(U-1), L2(U-1) | E2(U-2), L3(U-2)) so PE never
blocks on evacuations. Constants are fused into one bf16 blob -> single
DMA; the ACT activation-table load is pre-triggered by a dummy ReLU.

build_nc(nrep=K, staggered=True) wraps the pipeline in a hardware loop
(tc.For_i with staggered reset) so K back-to-back executions can be
timed in one dispatch - this is how test.py measures HW exec time under
the ~51ms axon RPC dispatch floor.
"""

import numpy as np
import ml_dtypes

import concourse.bass as bass
import concourse.bacc as bacc
import concourse.mybir as mybir
import concourse.tile as tile
from concourse.bass_utils import run_bass_kernel_spmd

# problem constants
N, D, H, S, B = 262144, 128, 128, 4, 1024
NCORES = 8

TS = 960             # atoms per compute tile (PSUM slot = 960 data cols
                     # + 64-col bank tail reserved for e-columns)
CHUNK = 2 * TS       # atoms per x load chunk (480 KB)
EREG = 64            # e-columns per parity region (the slot bank tails)

# constant blob layout (bf16, [128, CB])
_W1_OFF = 0
_W2_OFF = 512
_W3_OFF = 1024
_B1_OFF = 1028
_B2_OFF = 1032
CB = 1036

F32 = mybir.dt.float32
I32 = mybir.dt.int32
BF16 = mybir.dt.bfloat16
AF = mybir.ActivationFunctionType
ALU = mybir.AluOpType


def plan(ngs):
    """Static schedule shared by build_nc (device emission) and the host
    (e-column ordering): 960-col units, per-unit e-chunk emissions with
    parity-matched regions, and drain events."""
    ngs = tuple(int(g) for g in ngs)
    symbase = [0]
    for g in ngs:
        assert g % 128 == 0
        symbase.append(symbase[-1] + g)
    NS = symbase[-1]
    KC = NS // 128
    units = []
    for s in range(S):
        off = 0
        while off < ngs[s]:
            sz = min(TS, ngs[s] - off)
            units.append((s, off, sz))
            off += sz
    # L3 for unit u is emitted at pipeline step u+3. Its e-columns go to
    # the ph2 slot bank tails with parity q=u%2 (the slot E2 is NOT
    # reading that step). Crucially the regions must NOT share banks
    # with ph1: ACT's E1 reads would pick up bank-granular dependencies
    # on L3 (PE) writes, closing a cycle E1->L2->L3->E1 that caps the
    # pipeline at ~1.7us/unit. With ph2 tails the analogous edge lands
    # on DVE's E2, whose FIFO is already the longer per-unit op, so it
    # stays latent. Each 64-col parity region is drained in 32-col
    # halves with a small delay so DVE never waits on in-flight L3
    # matmuls; reuse of a half comes ~8 units after its fill.
    HALF = EREG // 2
    DRAIN_DELAY = 2
    DMA_DELAY = 2    # e-output DMA lags its drain copy so the SP queue
                     # (shared with x-chunk loads) never blocks on DVE
    # Columns past the last full half per parity ("stubs") go to the
    # (otherwise unused) ph1 bank tails with parity (u+1)%2, so the
    # final drains read regions the next For_i iteration's early L3
    # writes never touch (no cross-iteration WAR putting the body tail
    # on the critical path); the E1<-L3 bank edge they create only
    # affects the next iteration's first E1s, at the body boundary.
    nfull = [0, 0]  # full halves per parity, counted in a dry run
    dry = [0, 0]
    drycomp = [0] * S
    for u, (s, goff, sz) in enumerate(units):
        p = u % 2
        while (drycomp[s] + 1) * 128 <= goff + sz:
            drycomp[s] += 1
            dry[p] += 1
    nfull = [dry[0] // HALF * HALF, dry[1] // HALF * HALF]

    completed = [0] * S
    cntP = [0, 0]
    cntQ = [0, 0]
    bufP = [[], []]  # gcols currently sitting in the filling half-region
    bufQ = [[], []]
    dpos = 0
    col_perm = []
    unit_events = [[] for _ in units]
    final_events = []
    for u, (s, goff, sz) in enumerate(units):
        ev = unit_events[u]
        p = u % 2
        covered = goff + sz
        while (completed[s] + 1) * 128 <= covered:
            j = completed[s]
            completed[s] += 1
            gcol = symbase[s] // 128 + j
            if cntP[p] < nfull[p]:
                roff = cntP[p] % EREG
                ev.append(("col", s, j, p, roff))
                bufP[p].append(gcol)
                cntP[p] += 1
                if cntP[p] % HALF == 0:
                    hs = (cntP[p] - HALF) % EREG
                    drain = ("drain", p, hs, HALF, dpos)
                    edma = ("edma", HALF, dpos)
                    if u + DRAIN_DELAY < len(units):
                        unit_events[u + DRAIN_DELAY].append(drain)
                    else:
                        final_events.append(drain)
                    if u + DRAIN_DELAY + DMA_DELAY < len(units):
                        unit_events[u + DRAIN_DELAY + DMA_DELAY].append(edma)
                    else:
                        final_events.append(edma)
                    col_perm.extend(bufP[p])
                    bufP[p] = []
                    dpos += HALF
            else:
                q = (u + 1) % 2
                ev.append(("scol", s, j, q, cntQ[q]))
                bufQ[q].append(gcol)
                cntQ[q] += 1
    for q in (0, 1):
        n = len(bufQ[q])
        if n:
            assert n <= EREG
            final_events.append(("sdrain", q, 0, n, dpos))
            final_events.append(("edma", n, dpos))
            col_perm.extend(bufQ[q])
            dpos += n
    assert dpos == KC and all(completed[s] * 128 == ngs[s] for s in range(S))
    return dict(
        ngs=ngs, symbase=symbase, NS=NS, KC=KC, units=units,
        unit_events=unit_events, final_events=final_events,
        col_perm=np.asarray(col_perm, np.int64),
    )


_LAST_NGS = None  # set by prepare_inputs; build_nc default


def build_nc(ngs=None, act_e2=None, nrep=1, unroll=1, staggered=False,
             dbg_no_l3=False, dbg_no_e2=False, dbg_l3_const_src=False,
             dbg_no_drain=False, lag=5, prefetch=2, gbufs=4):
    if ngs is None:
        ngs = _LAST_NGS
    assert ngs is not None, "call prepare_inputs first or pass ngs"
    pl = plan(ngs)
    NS, KC = pl["NS"], pl["KC"]
    units, symbase = pl["units"], pl["symbase"]
    NU = len(units)
    if act_e2 is None:
        # stage-2 evacuations moved from DVE to ACT for engine balance
        act_e2 = frozenset((9, 19, 29))
    act_e2 = frozenset(act_e2)

    nc = bacc.Bacc()

    xst_d = nc.declare_dram_parameter("xst", [D, NS], BF16, isOutput=False)
    cst_d = nc.declare_dram_parameter("cst", [128, CB], BF16, isOutput=False)
    e_d = nc.declare_dram_parameter("e", [128, KC], F32, isOutput=True)

    with tile.TileContext(nc) as tc:
        with (
            tc.tile_pool(name="const", bufs=1) as cpool,
            tc.tile_pool(name="xload", bufs=gbufs) as gpool,
            tc.tile_pool(name="h1", bufs=4) as h1pool,
            tc.tile_pool(name="psum", bufs=1, space="PSUM") as ppool,
        ):
            # ---- ACT table preload: dummy ReLU on a zeroed tile ----
            zt = cpool.tile([128, 1], F32, tag="zt")
            nc.vector.memset(zt[:], 0.0)
            zt2 = cpool.tile([128, 1], F32, tag="zt2")
            nc.scalar.activation(out=zt2[:], in_=zt[:], func=AF.Relu)

            # ---- preload constants: one bf16 DMA ----
            cst_sb = cpool.tile([128, CB], BF16, tag="cst")
            nc.sync.dma_start(out=cst_sb[:], in_=cst_d[:])
            w1_sb = [
                cst_sb[:, _W1_OFF + 128 * s : _W1_OFF + 128 * (s + 1)]
                for s in range(S)
            ]
            w2_sb = [
                cst_sb[:, _W2_OFF + 128 * s : _W2_OFF + 128 * (s + 1)]
                for s in range(S)
            ]
            w3_sb = [cst_sb[:, _W3_OFF + s : _W3_OFF + s + 1] for s in range(S)]
            b1f = cpool.tile([128, S], F32, tag="b1f")
            nc.vector.tensor_copy(
                out=b1f[:], in_=cst_sb[:, _B1_OFF : _B1_OFF + S]
            )
            b2f = cpool.tile([128, S], F32, tag="b2f")
            nc.vector.tensor_copy(
                out=b2f[:], in_=cst_sb[:, _B2_OFF : _B2_OFF + S]
            )
            b1_sb = [b1f[:, s : s + 1] for s in range(S)]
            b2_sb = [b2f[:, s : s + 1] for s in range(S)]

            # h2 ring: full-size so the 128-atom L3 chunks are contiguous
            # even though they straddle 960-col stage-2 tiles
            h2ring = cpool.tile([128, NS], BF16, tag="h2ring")
            e_sb = cpool.tile([128, KC], F32, tag="e_sb")

            # PSUM: one [128,4096] f32 tile = all 8 banks, sliced into
            # bank-aligned slots so concurrent PE writes and ACT/DVE reads
            # never share a bank:
            #  banks 0-1: ph1 slot0 [0:960]   + e-region parity0 [960:1024]
            #  banks 2-3: ph1 slot1 [1024:1984] + e-region parity1 [1984:2048]
            #  banks 4-5: ph2 slot0 [2048:3008]
            #  banks 6-7: ph2 slot1 [3072:4032]
            P = ppool.tile([128, 4096], F32, tag="P")
            ph1 = [P[:, 0:960], P[:, 1024:1984]]
            sreg = [P[:, 960:1024], P[:, 1984:2048]]   # stub e-columns
            ph2 = [P[:, 2048:3008], P[:, 3072:4032]]
            ereg = [P[:, 3008:3072], P[:, 4032:4096]]  # main e-columns

            def spans(size):
                # per-slot matmul splits at the bank boundary (col 512)
                if size <= 512:
                    return [(0, size)]
                return [(0, 512), (512, size)]

            def evac(eng, out, in_, bias):
                if eng == "act":
                    nc.scalar.activation(
                        out=out, in_=in_, func=AF.Relu, bias=bias
                    )
                else:
                    nc.vector.tensor_scalar(
                        out=out, in0=in_, scalar1=bias, scalar2=0.0,
                        op0=ALU.add, op1=ALU.max,
                    )

            def body():
                xch = {}
                h1_u = {}

                def load_chunk(s, ci):
                    if (s, ci) in xch:
                        return
                    base = symbase[s] + ci * CHUNK
                    sz = min(CHUNK, ngs[s] - ci * CHUNK)
                    xt = gpool.tile([128, CHUNK], BF16, tag="xtc")
                    nc.sync.dma_start(
                        out=xt[:, :sz], in_=xst_d[:, base : base + sz]
                    )
                    xch[(s, ci)] = xt

                # L3 lags E2 by LAG-2 steps so its DVE dependency is
                # always satisfied when the in-order PE stream reaches it
                # (a blocked L3 would stall every later PE op and starve
                # ACT). LAG must be odd to keep the e-region parity rule.
                LAG = lag
                for T in range(NU + LAG):
                    # prefetch the x chunk two units ahead so the SP queue
                    # stays in front of the PE
                    if prefetch and T + prefetch < NU:
                        s2, goff2, _ = units[T + prefetch]
                        load_chunk(s2, goff2 // CHUNK)
                    # L1 for unit T
                    if T < NU:
                        s, goff, sz = units[T]
                        ci, co = divmod(goff, CHUNK)
                        load_chunk(s, ci)
                        slot = ph1[T % 2]
                        for c0, c1 in spans(sz):
                            nc.tensor.matmul(
                                out=slot[:, c0:c1], lhsT=w1_sb[s],
                                rhs=xch[(s, ci)][:, co + c0 : co + c1],
                                start=True, stop=True,
                            )
                    # E1 + L2 for unit T-1
                    U = T - 1
                    if 0 <= U < NU:
                        s, goff, sz = units[U]
                        h1_sb = h1pool.tile([128, TS], BF16, tag="h1_sb")
                        evac("act", h1_sb[:, :sz], ph1[U % 2][:, :sz], b1_sb[s])
                        h1_u[U] = h1_sb
                        slot = ph2[U % 2]
                        for c0, c1 in spans(sz):
                            nc.tensor.matmul(
                                out=slot[:, c0:c1], lhsT=w2_sb[s],
                                rhs=h1_sb[:, c0:c1],
                                start=True, stop=True,
                            )
                    # E2 for unit T-2
                    U = T - 2
                    if 0 <= U < NU:
                        s, goff, sz = units[U]
                        gflat = symbase[s] + goff
                        h1_u.pop(U, None)
                        if not dbg_no_e2:
                            evac(
                                "act" if U in act_e2 else "dve",
                                h2ring[:, gflat : gflat + sz],
                                ph2[U % 2][:, :sz], b2_sb[s],
                            )
                    # L3 + drains for unit T-LAG
                    U = T - LAG
                    if 0 <= U < NU and not dbg_no_l3:
                        for ev in pl["unit_events"][U]:
                            if ev[0] in ("col", "scol"):
                                _, es, j, p, roff = ev
                                reg = sreg if ev[0] == "scol" else ereg
                                lhsT = (
                                    cst_sb[:, 0:128] if dbg_l3_const_src
                                    else h2ring[
                                        :, symbase[es] + 128 * j :
                                        symbase[es] + 128 * (j + 1)
                                    ]
                                )
                                nc.tensor.matmul(
                                    out=reg[p][:, roff : roff + 1],
                                    lhsT=lhsT,
                                    rhs=w3_sb[es],
                                    start=True, stop=True,
                                )
                            elif ev[0] == "edma":
                                if not dbg_no_drain:
                                    _, n, dpos = ev
                                    nc.sync.dma_start(
                                        out=e_d[:, dpos : dpos + n],
                                        in_=e_sb[:, dpos : dpos + n],
                                    )
                            elif not dbg_no_drain:
                                _, p, hs, n, dpos = ev
                                reg = sreg if ev[0] == "sdrain" else ereg
                                nc.vector.tensor_copy(
                                    out=e_sb[:, dpos : dpos + n],
                                    in_=reg[p][:, hs : hs + n],
                                )
                for ev in ([] if dbg_no_l3 else pl["final_events"]):
                    if ev[0] == "edma":
                        _, n, dpos = ev
                        nc.sync.dma_start(
                            out=e_d[:, dpos : dpos + n],
                            in_=e_sb[:, dpos : dpos + n],
                        )
                    else:
                        _, p, hs, n, dpos = ev
                        reg = sreg if ev[0] == "sdrain" else ereg
                        nc.vector.tensor_copy(
                            out=e_sb[:, dpos : dpos + n],
                            in_=reg[p][:, hs : hs + n],
                        )

            if nrep == 1:
                body()
            else:
                assert nrep % unroll == 0
                with tc.For_i(0, nrep // unroll, 1, staggered_reset=staggered):
                    for _ in range(unroll):
                        body()
    nc.finalize()
    return nc


def prepare_inputs(x, symbol_ids, image_ids, W1, b1, W2, b2, W3, b3, slope,
                   intercept):
    """Global stable sort by symbol, equal per-(core,symbol) dealing;
    run-boundary tables kept host-side. Returns (in_maps, metas)."""
    global _LAST_NGS
    x = np.ascontiguousarray(np.asarray(x, dtype=np.float32))
    sym = np.asarray(symbol_ids, dtype=np.int32)
    img = np.asarray(image_ids, dtype=np.int32)
    W1 = np.ascontiguousarray(np.asarray(W1, np.float32))
    W2 = np.ascontiguousarray(np.asarray(W2, np.float32))
    W3 = np.asarray(W3, np.float32)
    b1 = np.ascontiguousarray(np.asarray(b1, np.float32))
    b2 = np.ascontiguousarray(np.asarray(b2, np.float32))
    b3 = np.asarray(b3, np.float32)
    slope = np.asarray(slope, np.float32)
    intercept = np.asarray(intercept, np.float32)

    W3c = (W3 * slope[:, None]).astype(np.float32)
    cvec = (slope * b3 + intercept).astype(np.float32).reshape(1, S)

    cst = np.zeros((128, CB), ml_dtypes.bfloat16)
    for s in range(S):
        cst[:, _W1_OFF + 128 * s : _W1_OFF + 128 * (s + 1)] = W1[s]
        cst[:, _W2_OFF + 128 * s : _W2_OFF + 128 * (s + 1)] = W2[s]
        cst[:, _W3_OFF + s] = W3c[s]
        cst[:, _B1_OFF + s] = b1[s]
        cst[:, _B2_OFF + s] = b2[s]

    order = np.argsort(sym, kind="stable").astype(np.int64)
    counts = np.bincount(sym, minlength=S)
    starts = np.concatenate([[0], np.cumsum(counts)])
    gs = [(int(counts[s]) + NCORES - 1) // NCORES for s in range(S)]
    ngs = tuple((g + 127) // 128 * 128 for g in gs)
    _LAST_NGS = ngs
    pl = plan(ngs)
    NS, symbase = pl["NS"], pl["symbase"]

    in_maps, metas = [], []
    for k in range(NCORES):
        xs = np.zeros((NS, D), ml_dtypes.bfloat16)
        bnd = np.zeros(S * (B + 1), np.int64)
        cnts = np.zeros((S, B), np.int64)
        for s in range(S):
            lo = starts[s] + k * gs[s]
            hi = min(starts[s] + (k + 1) * gs[s], starts[s + 1])
            gidx = order[lo:hi]
            cnt = hi - lo
            base = symbase[s]
            xs[base : base + cnt] = x[gidx]
            gimg = img[gidx]
            ends = np.searchsorted(gimg, np.arange(B), "right")
            bnd[s * (B + 1) : s * (B + 1) + B] = base + ends - 1
            bnd[s * (B + 1) + B] = base + ngs[s] - 1
            cnts[s] = np.diff(np.concatenate([[0], ends]))
        xst = np.ascontiguousarray(xs.T)  # [D, NS] bf16
        in_maps.append(dict(xst=xst, cst=cst))
        metas.append((bnd, cnts, cvec))
    return in_maps, (metas, pl)


def finish_output(results, metas):
    """Per-image energies from device per-atom energies: host prefix sums +
    O(B) boundary diffs."""
    metas, pl = metas
    NS, KC = pl["NS"], pl["KC"]
    col_perm = pl["col_perm"]
    out = np.zeros(B, np.float32)
    for k in range(NCORES):
        bnd, cnts, cvec = metas[k]
        e2d = np.asarray(results[k]["e"], np.float64)  # [128, KC]
        e_flat = np.zeros((KC, 128), np.float64)
        e_flat[col_perm] = e2d.T
        gp = np.cumsum(e_flat.reshape(-1))
        q = bnd
        gpv = np.where(q >= 0, gp[np.maximum(q, 0)], 0.0)
        t = np.concatenate([[0.0], gpv])
        rs = (t[1:] - t[:-1]).reshape(S, B + 1)[:, :B]
        rs = rs + cvec.reshape(S, 1) * cnts  # per-symbol affine constants
        out += rs.sum(axis=0).astype(np.float32)
    return out


_NC_CACHE = {}


def kernel(**inputs):
    in_maps, metas = prepare_inputs(**inputs)
    ngs = metas[1]["ngs"]
    if ngs not in _NC_CACHE:
        _NC_CACHE[ngs] = build_nc(ngs)
    res = run_bass_kernel_spmd(_NC_CACHE[ngs], in_maps, list(range(NCORES)))
    return finish_output(res.results, metas)


# revision 38
# speedup vs baseline: 1.1329x; 1.0489x over previous
"""Trainium2 Bass kernel for nn_AutoEncoder_53781580481200 (moe_routing).

Host/device split:
  host: atoms are globally stable-sorted by symbol (the MoE routing) and
        dealt to the 8 cores in equal per-(core,symbol) slices, so every
        core runs an identical program with minimal padding (NG_s =
        ceil(ceil(C_s/8)/128)*128 per symbol, chosen at runtime from the
        data); x is stored transposed [D, NS] in bf16. Per-(core,symbol,
        image) run-boundary tables stay host-side.
  device (per core): per-symbol 2-layer MLP + energy head, matmuls bf16.
        The hard floor is PSUM evacuation: only ACT and DVE can read
        PSUM, both at 1 elem/cycle/lane for f32 sources (ACT 1.2 GHz,
        DVE 0.96 GHz), so the two ReLU+bias stages (2*NS columns total)
        are split across them, balanced by moving a few stage-2 tiles to
        ACT. Uniform 960-column tiling gives one evacuation op per tile
        per stage (ACT op = FD+222 cy, DVE op = FD+120 cy) while fitting
        PSUM: one [128,4096] f32 tile sliced into four bank-aligned
        960-col slots (L1/L2 ping-pong) whose 64-col bank tails hold the
        e-columns. L3 writes each 128-atom energy column (lhsT = h2
        chunk, rhs = w3*slope) into the tail of the slot parity it just
        drained, so PE writes never share a PSUM bank with a concurrent
        ACT/DVE read (fatal on TRN2); filled 64-col regions are drained
        to SBUF/DRAM as they complete.
  host: per-core cumsum of the (reordered) per-atom energies; per-image
        energies = prefix diffs at run boundaries + per-symbol affine
        constants x run counts (O(B)).

The pipeline is software-pipelined over 960-atom units: emission order
skews stages (L1(U) | E1(U-1), L2(U-1) | E2(U-2), L3(U-2)) so PE never
blocks on evacuations. Constants are fused into one bf16 blob -> single
DMA; the ACT activation-table load is pre-triggered by a dummy ReLU.

build_nc(nrep=K, staggered=True) wraps the pipeline in a hardware loop
(tc.For_i with staggered reset) so K back-to-back executions can be
timed in one dispatch - this is how test.py measures HW exec time under
the ~51ms axon RPC dispatch floor.
"""

import numpy as np
import ml_dtypes

import concourse.bass as bass
import concourse.bacc as bacc
import concourse.mybir as mybir
import concourse.tile as tile
from concourse.bass_utils import run_bass_kernel_spmd

# problem constants
N, D, H, S, B = 262144, 128, 128, 4, 1024
NCORES = 8

TS = 960             # atoms per compute tile (PSUM slot = 960 data cols
                     # + 64-col bank tail reserved for e-columns)
CHUNK = 2 * TS       # atoms per x load chunk (480 KB)
EREG = 64            # e-columns per parity region (the slot bank tails)

# constant blob layout (bf16, [128, CB])
_W1_OFF = 0
_W2_OFF = 512
_W3_OFF = 1024
_B1_OFF = 1028
_B2_OFF = 1032
CB = 1036

F32 = mybir.dt.float32
I32 = mybir.dt.int32
BF16 = mybir.dt.bfloat16
AF = mybir.ActivationFunctionType
ALU = mybir.AluOpType


def plan(ngs):
    """Static schedule shared by build_nc (device emission) and the host
    (e-column ordering): 960-col units, per-unit e-chunk emissions with
    parity-matched regions, and drain events."""
    ngs = tuple(int(g) for g in ngs)
    symbase = [0]
    for g in ngs:
        assert g % 128 == 0
        symbase.append(symbase[-1] + g)
    NS = symbase[-1]
    KC = NS // 128
    units = []
    for s in range(S):
        off = 0
        while off < ngs[s]:
            sz = min(TS, ngs[s] - off)
            units.append((s, off, sz))
            off += sz
    # L3 for unit u is emitted at pipeline step u+3. Its e-columns go to
    # the ph2 slot bank tails with parity q=u%2 (the slot E2 is NOT
    # reading that step). Crucially the regions must NOT share banks
    # with ph1: ACT's E1 reads would pick up bank-granular dependencies
    # on L3 (PE) writes, closing a cycle E1->L2->L3->E1 that caps the
    # pipeline at ~1.7us/unit. With ph2 tails the analogous edge lands
    # on DVE's E2, whose FIFO is already the longer per-unit op, so it
    # stays latent. Each 64-col parity region is drained in 32-col
    # halves with a small delay so DVE never waits on in-flight L3
    # matmuls; reuse of a half comes ~8 units after its fill.
    HALF = EREG // 2
    DRAIN_DELAY = 2
    DMA_DELAY = 2    # e-output DMA lags its drain copy so the SP queue
                     # (shared with x-chunk loads) never blocks on DVE
    # Drains that would land past the last unit are DEFERRED: the body
    # emits them at its START (reading the e-regions left over from the
    # previous For_i iteration, when DVE is idle and the deps are long
    # satisfied), and the same drains are emitted once more after the
    # loop for the final iteration. This keeps the body tail off the
    # cross-iteration critical path entirely.
    completed = [0] * S
    cntP = [0, 0]
    bufP = [[], []]  # gcols currently sitting in the filling half-region
    dpos = 0
    col_perm = []
    unit_events = [[] for _ in units]
    final_events = []

    def emit_drain(u, p, hs, n):
        nonlocal dpos
        drain = ("drain", p, hs, n, dpos)
        edma = ("edma", n, dpos)
        if u is not None and u + DRAIN_DELAY < len(units):
            unit_events[u + DRAIN_DELAY].append(drain)
        else:
            final_events.append(drain)
        if u is not None and u + DRAIN_DELAY + DMA_DELAY < len(units):
            unit_events[u + DRAIN_DELAY + DMA_DELAY].append(edma)
        else:
            final_events.append(edma)
        col_perm.extend(bufP[p])
        bufP[p] = []
        dpos += n

    for u, (s, goff, sz) in enumerate(units):
        ev = unit_events[u]
        p = u % 2
        covered = goff + sz
        while (completed[s] + 1) * 128 <= covered:
            j = completed[s]
            completed[s] += 1
            ev.append(("col", s, j, p, cntP[p] % EREG))
            bufP[p].append(symbase[s] // 128 + j)
            cntP[p] += 1
            if cntP[p] % HALF == 0:
                emit_drain(u, p, (cntP[p] - HALF) % EREG, HALF)
    for p in (0, 1):
        n = cntP[p] % HALF
        if n:
            emit_drain(None, p, (cntP[p] - n) % EREG, n)
    assert dpos == KC and all(completed[s] * 128 == ngs[s] for s in range(S))
    return dict(
        ngs=ngs, symbase=symbase, NS=NS, KC=KC, units=units,
        unit_events=unit_events, final_events=final_events,
        col_perm=np.asarray(col_perm, np.int64),
    )


_LAST_NGS = None  # set by prepare_inputs; build_nc default


def build_nc(ngs=None, act_e2=None, nrep=1, unroll=1, staggered=False,
             dbg_no_l3=False, dbg_no_e2=False, dbg_l3_const_src=False,
             dbg_no_drain=False, lag=5, prefetch=2, gbufs=4):
    if ngs is None:
        ngs = _LAST_NGS
    assert ngs is not None, "call prepare_inputs first or pass ngs"
    pl = plan(ngs)
    NS, KC = pl["NS"], pl["KC"]
    units, symbase = pl["units"], pl["symbase"]
    NU = len(units)
    if act_e2 is None:
        # stage-2 evacuations moved from DVE to ACT for engine balance
        act_e2 = frozenset((9, 19, 29))
    act_e2 = frozenset(act_e2)

    nc = bacc.Bacc()

    xst_d = nc.declare_dram_parameter("xst", [D, NS], BF16, isOutput=False)
    cst_d = nc.declare_dram_parameter("cst", [128, CB], BF16, isOutput=False)
    e_d = nc.declare_dram_parameter("e", [128, KC], F32, isOutput=True)

    with tile.TileContext(nc) as tc:
        with (
            tc.tile_pool(name="const", bufs=1) as cpool,
            tc.tile_pool(name="xload", bufs=gbufs) as gpool,
            tc.tile_pool(name="h1", bufs=4) as h1pool,
            tc.tile_pool(name="psum", bufs=1, space="PSUM") as ppool,
        ):
            # ---- ACT table preload: dummy ReLU on a zeroed tile ----
            zt = cpool.tile([128, 1], F32, tag="zt")
            nc.vector.memset(zt[:], 0.0)
            zt2 = cpool.tile([128, 1], F32, tag="zt2")
            nc.scalar.activation(out=zt2[:], in_=zt[:], func=AF.Relu)

            # ---- preload constants: one bf16 DMA ----
            cst_sb = cpool.tile([128, CB], BF16, tag="cst")
            nc.sync.dma_start(out=cst_sb[:], in_=cst_d[:])
            w1_sb = [
                cst_sb[:, _W1_OFF + 128 * s : _W1_OFF + 128 * (s + 1)]
                for s in range(S)
            ]
            w2_sb = [
                cst_sb[:, _W2_OFF + 128 * s : _W2_OFF + 128 * (s + 1)]
                for s in range(S)
            ]
            w3_sb = [cst_sb[:, _W3_OFF + s : _W3_OFF + s + 1] for s in range(S)]
            b1f = cpool.tile([128, S], F32, tag="b1f")
            nc.vector.tensor_copy(
                out=b1f[:], in_=cst_sb[:, _B1_OFF : _B1_OFF + S]
            )
            b2f = cpool.tile([128, S], F32, tag="b2f")
            nc.vector.tensor_copy(
                out=b2f[:], in_=cst_sb[:, _B2_OFF : _B2_OFF + S]
            )
            b1_sb = [b1f[:, s : s + 1] for s in range(S)]
            b2_sb = [b2f[:, s : s + 1] for s in range(S)]

            # h2 ring: full-size so the 128-atom L3 chunks are contiguous
            # even though they straddle 960-col stage-2 tiles
            h2ring = cpool.tile([128, NS], BF16, tag="h2ring")
            e_sb = cpool.tile([128, KC], F32, tag="e_sb")

            # PSUM: one [128,4096] f32 tile = all 8 banks, sliced into
            # bank-aligned slots so concurrent PE writes and ACT/DVE reads
            # never share a bank:
            #  banks 0-1: ph1 slot0 [0:960]   + e-region parity0 [960:1024]
            #  banks 2-3: ph1 slot1 [1024:1984] + e-region parity1 [1984:2048]
            #  banks 4-5: ph2 slot0 [2048:3008]
            #  banks 6-7: ph2 slot1 [3072:4032]
            P = ppool.tile([128, 4096], F32, tag="P")
            ph1 = [P[:, 0:960], P[:, 1024:1984]]
            ph2 = [P[:, 2048:3008], P[:, 3072:4032]]
            ereg = [P[:, 3008:3072], P[:, 4032:4096]]  # e-columns
            # establish the e-regions as written so the first body's
            # carry drains (which read the previous iteration's regions)
            # are legal on iteration 0
            nc.vector.memset(ereg[0][:], 0.0)
            nc.vector.memset(ereg[1][:], 0.0)

            def spans(size):
                # per-slot matmul splits at the bank boundary (col 512)
                if size <= 512:
                    return [(0, size)]
                return [(0, 512), (512, size)]

            def evac(eng, out, in_, bias):
                if eng == "act":
                    nc.scalar.activation(
                        out=out, in_=in_, func=AF.Relu, bias=bias
                    )
                else:
                    nc.vector.tensor_scalar(
                        out=out, in0=in_, scalar1=bias, scalar2=0.0,
                        op0=ALU.add, op1=ALU.max,
                    )

            def emit_drains(evs):
                for ev in evs:
                    if ev[0] == "edma":
                        _, n, dpos = ev
                        nc.sync.dma_start(
                            out=e_d[:, dpos : dpos + n],
                            in_=e_sb[:, dpos : dpos + n],
                        )
                    else:
                        _, p, hs, n, dpos = ev
                        nc.vector.tensor_copy(
                            out=e_sb[:, dpos : dpos + n],
                            in_=ereg[p][:, hs : hs + n],
                        )

            def body():
                xch = {}
                h1_u = {}

                def load_chunk(s, ci):
                    if (s, ci) in xch:
                        return
                    base = symbase[s] + ci * CHUNK
                    sz = min(CHUNK, ngs[s] - ci * CHUNK)
                    xt = gpool.tile([128, CHUNK], BF16, tag="xtc")
                    nc.sync.dma_start(
                        out=xt[:, :sz], in_=xst_d[:, base : base + sz]
                    )
                    xch[(s, ci)] = xt

                # carry: drain the e-region tails left by the PREVIOUS
                # body while this body's pipeline fills (DVE idle, deps
                # satisfied) - keeps the body tail off the critical path
                if not dbg_no_l3 and not dbg_no_drain:
                    emit_drains(pl["final_events"])

                # L3 lags E2 by LAG-2 steps so its DVE dependency is
                # always satisfied when the in-order PE stream reaches it
                # (a blocked L3 would stall every later PE op and starve
                # ACT). LAG must be odd to keep the e-region parity rule.
                LAG = lag
                for T in range(NU + LAG):
                    # prefetch the x chunk two units ahead so the SP queue
                    # stays in front of the PE
                    if prefetch and T + prefetch < NU:
                        s2, goff2, _ = units[T + prefetch]
                        load_chunk(s2, goff2 // CHUNK)
                    # L1 for unit T
                    if T < NU:
                        s, goff, sz = units[T]
                        ci, co = divmod(goff, CHUNK)
                        load_chunk(s, ci)
                        slot = ph1[T % 2]
                        for c0, c1 in spans(sz):
                            nc.tensor.matmul(
                                out=slot[:, c0:c1], lhsT=w1_sb[s],
                                rhs=xch[(s, ci)][:, co + c0 : co + c1],
                                start=True, stop=True,
                            )
                    # E1 + L2 for unit T-1
                    U = T - 1
                    if 0 <= U < NU:
                        s, goff, sz = units[U]
                        h1_sb = h1pool.tile([128, TS], BF16, tag="h1_sb")
                        evac("act", h1_sb[:, :sz], ph1[U % 2][:, :sz], b1_sb[s])
                        h1_u[U] = h1_sb
                        slot = ph2[U % 2]
                        for c0, c1 in spans(sz):
                            nc.tensor.matmul(
                                out=slot[:, c0:c1], lhsT=w2_sb[s],
                                rhs=h1_sb[:, c0:c1],
                                start=True, stop=True,
                            )
                    # E2 for unit T-2
                    U = T - 2
                    if 0 <= U < NU:
                        s, goff, sz = units[U]
                        gflat = symbase[s] + goff
                        h1_u.pop(U, None)
                        if not dbg_no_e2:
                            evac(
                                "act" if U in act_e2 else "dve",
                                h2ring[:, gflat : gflat + sz],
                                ph2[U % 2][:, :sz], b2_sb[s],
                            )
                    # L3 + drains for unit T-LAG
                    U = T - LAG
                    if 0 <= U < NU and not dbg_no_l3:
                        for ev in pl["unit_events"][U]:
                            if ev[0] == "col":
                                _, es, j, p, roff = ev
                                lhsT = (
                                    cst_sb[:, 0:128] if dbg_l3_const_src
                                    else h2ring[
                                        :, symbase[es] + 128 * j :
                                        symbase[es] + 128 * (j + 1)
                                    ]
                                )
                                nc.tensor.matmul(
                                    out=ereg[p][:, roff : roff + 1],
                                    lhsT=lhsT,
                                    rhs=w3_sb[es],
                                    start=True, stop=True,
                                )
                            elif ev[0] == "edma":
                                if not dbg_no_drain:
                                    _, n, dpos = ev
                                    nc.sync.dma_start(
                                        out=e_d[:, dpos : dpos + n],
                                        in_=e_sb[:, dpos : dpos + n],
                                    )
                            elif not dbg_no_drain:
                                _, p, hs, n, dpos = ev
                                nc.vector.tensor_copy(
                                    out=e_sb[:, dpos : dpos + n],
                                    in_=ereg[p][:, hs : hs + n],
                                )

            if nrep == 1:
                body()
            elif nrep == unroll:
                for _ in range(nrep):
                    body()
            else:
                assert nrep % unroll == 0
                with tc.For_i(0, nrep // unroll, 1, staggered_reset=staggered):
                    for _ in range(unroll):
                        body()
            # final iteration's deferred tail drains
            if not dbg_no_l3 and not dbg_no_drain:
                emit_drains(pl["final_events"])
    nc.finalize()
    return nc


def prepare_inputs(x, symbol_ids, image_ids, W1, b1, W2, b2, W3, b3, slope,
                   intercept):
    """Global stable sort by symbol, equal per-(core,symbol) dealing;
    run-boundary tables kept host-side. Returns (in_maps, metas)."""
    global _LAST_NGS
    x = np.ascontiguousarray(np.asarray(x, dtype=np.float32))
    sym = np.asarray(symbol_ids, dtype=np.int32)
    img = np.asarray(image_ids, dtype=np.int32)
    W1 = np.ascontiguousarray(np.asarray(W1, np.float32))
    W2 = np.ascontiguousarray(np.asarray(W2, np.float32))
    W3 = np.asarray(W3, np.float32)
    b1 = np.ascontiguousarray(np.asarray(b1, np.float32))
    b2 = np.ascontiguousarray(np.asarray(b2, np.float32))
    b3 = np.asarray(b3, np.float32)
    slope = np.asarray(slope, np.float32)
    intercept = np.asarray(intercept, np.float32)

    W3c = (W3 * slope[:, None]).astype(np.float32)
    cvec = (slope * b3 + intercept).astype(np.float32).reshape(1, S)

    cst = np.zeros((128, CB), ml_dtypes.bfloat16)
    for s in range(S):
        cst[:, _W1_OFF + 128 * s : _W1_OFF + 128 * (s + 1)] = W1[s]
        cst[:, _W2_OFF + 128 * s : _W2_OFF + 128 * (s + 1)] = W2[s]
        cst[:, _W3_OFF + s] = W3c[s]
        cst[:, _B1_OFF + s] = b1[s]
        cst[:, _B2_OFF + s] = b2[s]

    order = np.argsort(sym, kind="stable").astype(np.int64)
    counts = np.bincount(sym, minlength=S)
    starts = np.concatenate([[0], np.cumsum(counts)])
    gs = [(int(counts[s]) + NCORES - 1) // NCORES for s in range(S)]
    ngs = tuple((g + 127) // 128 * 128 for g in gs)
    _LAST_NGS = ngs
    pl = plan(ngs)
    NS, symbase = pl["NS"], pl["symbase"]

    in_maps, metas = [], []
    for k in range(NCORES):
        xs = np.zeros((NS, D), ml_dtypes.bfloat16)
        bnd = np.zeros(S * (B + 1), np.int64)
        cnts = np.zeros((S, B), np.int64)
        for s in range(S):
            lo = starts[s] + k * gs[s]
            hi = min(starts[s] + (k + 1) * gs[s], starts[s + 1])
            gidx = order[lo:hi]
            cnt = hi - lo
            base = symbase[s]
            xs[base : base + cnt] = x[gidx]
            gimg = img[gidx]
            ends = np.searchsorted(gimg, np.arange(B), "right")
            bnd[s * (B + 1) : s * (B + 1) + B] = base + ends - 1
            bnd[s * (B + 1) + B] = base + ngs[s] - 1
            cnts[s] = np.diff(np.concatenate([[0], ends]))
        xst = np.ascontiguousarray(xs.T)  # [D, NS] bf16
        in_maps.append(dict(xst=xst, cst=cst))
        metas.append((bnd, cnts, cvec))
    return in_maps, (metas, pl)


def finish_output(results, metas):
    """Per-image energies from device per-atom energies: host prefix sums +
    O(B) boundary diffs."""
    metas, pl = metas
    NS, KC = pl["NS"], pl["KC"]
    col_perm = pl["col_perm"]
    out = np.zeros(B, np.float32)
    for k in range(NCORES):
        bnd, cnts, cvec = metas[k]
        e2d = np.asarray(results[k]["e"], np.float64)  # [128, KC]
        e_flat = np.zeros((KC, 128), np.float64)
        e_flat[col_perm] = e2d.T
        gp = np.cumsum(e_flat.reshape(-1))
        q = bnd
        gpv = np.where(q >= 0, gp[np.maximum(q, 0)], 0.0)
        t = np.concatenate([[0.0], gpv])
        rs = (t[1:] - t[:-1]).reshape(S, B + 1)[:, :B]
        rs = rs + cvec.reshape(S, 1) * cnts  # per-symbol affine constants
        out += rs.sum(axis=0).astype(np.float32)
    return out


_NC_CACHE = {}


def kernel(**inputs):
    in_maps, metas = prepare_inputs(**inputs)
    ngs = metas[1]["ngs"]
    if ngs not in _NC_CACHE:
        _NC_CACHE[ngs] = build_nc(ngs)
    res = run_bass_kernel_spmd(_NC_CACHE[ngs], in_maps, list(range(NCORES)))
    return finish_output(res.results, metas)


# revision 45
# speedup vs baseline: 1.3010x; 1.1484x over previous
"""Trainium2 Bass kernel for nn_AutoEncoder_53781580481200 (moe_routing).

Host/device split:
  host: atoms are globally stable-sorted by symbol (the MoE routing) and
        dealt to the 8 cores in equal per-(core,symbol) slices, so every
        core runs an identical program with minimal padding (NG_s =
        ceil(ceil(C_s/8)/128)*128 per symbol, chosen at runtime from the
        data - ~2.3% less work than image-aligned sharding); x is stored
        transposed [D, NS] in bf16 (contiguous DMA rows, half the HBM
        traffic of f32). Per-(core,symbol,image) run-boundary tables
        stay host-side.
  device (per core): per-symbol 2-layer MLP + energy head, all matmuls
        bf16 at full PE rate. ReLU+bias evacuations are the true
        bottleneck (only ACT and DVE can read PSUM on TRN2; GPSIMD
        cannot, and matmul can't write 16-bit PSUM before TRN3), so the
        two stages are balanced across them: E1 (h1 = relu(W1.T x + b1))
        as per-tile ACT ops, E2 (h2) as one [128,1024] DVE op per pair
        of tiles. Energies accumulate as PSUM columns e[m,c] =
        e(atom c*128+m) via 128-column L3 matmuls (lhsT=h2 chunk,
        rhs=w3*slope) into a dedicated PSUM bank (no bank is ever
        shared between a PE write and a concurrent ACT/DVE read, which
        costs serializing semaphores on TRN2).
  host: gp = cumsum(e); per-image energies = prefix diffs at run
        boundaries + per-symbol affine constants x run counts (O(B)).

The pipeline is software-pipelined over units (pairs of tiles): engines
execute their streams in order, so the emission order skews stages
(L1(U) | E1(U-1), L2(U-1) | E2(U-2), L3(U-2)) to keep PE from blocking
on evacuations. Constants are fused into one bf16 blob -> single DMA;
the ACT activation-table load is pre-triggered by a dummy ReLU.

e_all is drained in two halves: the first mid-body once its columns are
final; the second is DEFERRED - each body drains its predecessor's tail
at body start (DVE idle, deps satisfied), and the loop is followed by
one post-loop drain for the final iteration, keeping the body tail off
the cross-iteration critical path.

build_nc(nrep=K, staggered=True) wraps the pipeline in a hardware loop
(tc.For_i with staggered reset, i.e. no full inter-iteration barrier)
so K back-to-back executions can be timed in one dispatch - this is how
test.py measures HW exec time under the ~51ms axon RPC dispatch floor.
"""

import numpy as np
import ml_dtypes

import concourse.bass as bass
import concourse.bacc as bacc
import concourse.mybir as mybir
import concourse.tile as tile
from concourse.bass_utils import run_bass_kernel_spmd

# problem constants
N, D, H, S, B = 262144, 128, 128, 4, 1024
NCORES = 8

T = 512              # atoms per full compute tile
CHUNK = 2048         # atoms per load chunk (512 KB)

# constant blob layout (bf16, [128, CB])
_W1_OFF = 0
_W2_OFF = 512
_W3_OFF = 1024
_B1_OFF = 1028
_B2_OFF = 1032
CB = 1036

F32 = mybir.dt.float32
I32 = mybir.dt.int32
BF16 = mybir.dt.bfloat16
AF = mybir.ActivationFunctionType
ALU = mybir.AluOpType


def plan(ngs):
    """Unit/e-column schedule shared by build_nc and the host. Units are
    pairs of 512-tiles (plus per-symbol remainder tiles); e-columns fill
    a dedicated PSUM bank in atom order."""
    ngs = tuple(int(g) for g in ngs)
    symbase = [0]
    for g in ngs:
        assert g % 128 == 0
        symbase.append(symbase[-1] + g)
    NS = symbase[-1]
    KC = NS // 128
    assert KC <= 512
    units = []
    col = 0
    for s in range(S):
        base = symbase[s]
        off = 0
        while ngs[s] - off >= 1024:
            t0 = (s, base + off, T, col)
            t1 = (s, base + off + T, T, col + 4)
            units.append((t0, t1))
            col += 8
            off += 1024
        rem = ngs[s] - off
        if rem > T:
            units.append(
                ((s, base + off, T, col), (s, base + off + T, rem - T, col + 4))
            )
            col += rem // 128
        elif rem:
            units.append(((s, base + off, rem, col),))
            col += rem // 128
    assert col == KC
    # first unit index by which all e-columns < KC//2 are emitted
    HC = KC // 2
    half_u = 0
    c = 0
    for u, unit in enumerate(units):
        c += sum(t[2] for t in unit) // 128
        if c >= HC:
            half_u = u
            break
    return dict(
        ngs=ngs, symbase=symbase, NS=NS, KC=KC, units=units, half_u=half_u,
    )


_LAST_NGS = None  # set by prepare_inputs; build_nc default


def build_nc(ngs=None, nrep=1, unroll=1, staggered=False):
    if ngs is None:
        ngs = _LAST_NGS
    assert ngs is not None, "call prepare_inputs first or pass ngs"
    pl = plan(ngs)
    NS, KC = pl["NS"], pl["KC"]
    units, symbase = pl["units"], pl["symbase"]
    NU = len(units)
    HC = KC // 2
    HALF_U = pl["half_u"]

    nc = bacc.Bacc()

    xst_d = nc.declare_dram_parameter("xst", [D, NS], BF16, isOutput=False)
    cst_d = nc.declare_dram_parameter("cst", [128, CB], BF16, isOutput=False)
    e_d = nc.declare_dram_parameter("e", [128, KC], F32, isOutput=True)

    with tile.TileContext(nc) as tc:
        with (
            tc.tile_pool(name="const", bufs=1) as cpool,
            tc.tile_pool(name="xload", bufs=4) as gpool,
            tc.tile_pool(name="h1", bufs=4) as h1pool,
            tc.tile_pool(name="h2", bufs=3) as h2pool,
            tc.tile_pool(name="seg", bufs=1) as spool,
            tc.tile_pool(name="ph1", bufs=3, space="PSUM") as ph1,
            tc.tile_pool(name="ph2", bufs=2, space="PSUM") as ph2,
            tc.tile_pool(name="pea", bufs=1, space="PSUM") as pea,
        ):
            # ---- ACT table preload: dummy ReLU on a zeroed tile ----
            zt = cpool.tile([128, 1], F32, tag="zt")
            nc.vector.memset(zt[:], 0.0)
            zt2 = cpool.tile([128, 1], F32, tag="zt2")
            nc.scalar.activation(out=zt2[:], in_=zt[:], func=AF.Relu)

            # ---- preload constants: one bf16 DMA ----
            cst_sb = cpool.tile([128, CB], BF16, tag="cst")
            nc.sync.dma_start(out=cst_sb[:], in_=cst_d[:])
            w1_sb = [
                cst_sb[:, _W1_OFF + 128 * s : _W1_OFF + 128 * (s + 1)]
                for s in range(S)
            ]
            w2_sb = [
                cst_sb[:, _W2_OFF + 128 * s : _W2_OFF + 128 * (s + 1)]
                for s in range(S)
            ]
            w3_sb = [cst_sb[:, _W3_OFF + s : _W3_OFF + s + 1] for s in range(S)]
            b1f = cpool.tile([128, S], F32, tag="b1f")
            nc.vector.tensor_copy(
                out=b1f[:], in_=cst_sb[:, _B1_OFF : _B1_OFF + S]
            )
            b2f = cpool.tile([128, S], F32, tag="b2f")
            nc.vector.tensor_copy(
                out=b2f[:], in_=cst_sb[:, _B2_OFF : _B2_OFF + S]
            )
            b1_sb = [b1f[:, s : s + 1] for s in range(S)]
            b2_sb = [b2f[:, s : s + 1] for s in range(S)]

            # e_all: persistent dedicated PSUM bank + SBUF staging
            e_all = pea.tile([128, KC], F32, tag="eall")
            nc.vector.memset(e_all[:], 0.0)
            e_sb = spool.tile([128, KC], F32, tag="e_sb")

            def evac(eng, out, in_, bias):
                if eng == "act":
                    nc.scalar.activation(
                        out=out, in_=in_, func=AF.Relu, bias=bias
                    )
                else:
                    nc.vector.tensor_scalar(
                        out=out, in0=in_, scalar1=bias, scalar2=0.0,
                        op0=ALU.add, op1=ALU.max,
                    )

            def drain_tail():
                # second e half: deferred to the next body / post-loop
                nc.vector.tensor_copy(out=e_sb[:, HC:], in_=e_all[:, HC:])
                nc.sync.dma_start(out=e_d[:, HC:], in_=e_sb[:, HC:])

            def body():
                h1_ps_u, h2_ps_u = {}, {}
                xch = {}

                def load_chunk(s, ci):
                    if (s, ci) in xch:
                        return
                    base = symbase[s] + ci * CHUNK
                    sz = min(CHUNK, ngs[s] - ci * CHUNK)
                    xt = gpool.tile([128, CHUNK], BF16, tag="xtc")
                    nc.sync.dma_start(
                        out=xt[:, :sz], in_=xst_d[:, base : base + sz]
                    )
                    xch[(s, ci)] = xt

                # carry: drain the predecessor body's e tail while this
                # body's pipeline fills
                drain_tail()

                for U in range(NU + 3):
                    # L1 for unit U
                    if U < NU:
                        tiles = []
                        for (s, off, sz, _c) in units[U]:
                            woff = off - symbase[s]
                            ci, co = divmod(woff, CHUNK)
                            load_chunk(s, ci)
                            h1_ps = ph1.tile([128, T], F32, tag="h1_ps")
                            nc.tensor.matmul(
                                out=h1_ps[:, :sz], lhsT=w1_sb[s],
                                rhs=xch[(s, ci)][:, co : co + sz],
                                start=True, stop=True,
                            )
                            tiles.append(h1_ps)
                        h1_ps_u[U] = tiles
                    # E1 + L2 for unit U-1 (E1 as ACT singles so L2 of the
                    # first tile starts while the second evacuates)
                    Um = U - 1
                    if 0 <= Um < NU:
                        unit = units[Um]
                        usz = sum(t[2] for t in unit)
                        h1_sb = h1pool.tile([128, 2 * T], BF16, tag="h1_sb")
                        h2_ps = ph2.tile([128, 2 * T], F32, tag="h2_ps")
                        lo = 0
                        for (s, off, sz, _c), h1_ps in zip(unit, h1_ps_u.pop(Um)):
                            evac(
                                "act", h1_sb[:, lo : lo + sz],
                                h1_ps[:, :sz], b1_sb[s],
                            )
                            nc.tensor.matmul(
                                out=h2_ps[:, lo : lo + sz], lhsT=w2_sb[s],
                                rhs=h1_sb[:, lo : lo + sz],
                                start=True, stop=True,
                            )
                            lo += sz
                        h2_ps_u[Um] = h2_ps
                    # E2 (one DVE op per unit) + L3 for unit U-2
                    Um = U - 2
                    if 0 <= Um < NU:
                        unit = units[Um]
                        usz = sum(t[2] for t in unit)
                        s0 = unit[0][0]
                        h2_sb = h2pool.tile([128, 2 * T], BF16, tag="h2_sb")
                        h2_ps = h2_ps_u.pop(Um)
                        evac("dve", h2_sb[:, :usz], h2_ps[:, :usz], b2_sb[s0])
                        lo = 0
                        for (s, off, sz, c0) in unit:
                            for j in range(sz // 128):
                                nc.tensor.matmul(
                                    out=e_all[:, c0 + j : c0 + j + 1],
                                    lhsT=h2_sb[:, lo + j * 128 : lo + (j + 1) * 128],
                                    rhs=w3_sb[s],
                                    start=True, stop=True,
                                )
                            lo += sz
                    # first-half e evacuation as soon as its columns final
                    if U == HALF_U + 4:
                        nc.vector.tensor_copy(
                            out=e_sb[:, :HC], in_=e_all[:, :HC]
                        )
                    if U == HALF_U + 6:
                        nc.sync.dma_start(out=e_d[:, :HC], in_=e_sb[:, :HC])

            if nrep == 1:
                body()
            elif nrep == unroll:
                for _ in range(nrep):
                    body()
            else:
                assert nrep % unroll == 0
                with tc.For_i(0, nrep // unroll, 1, staggered_reset=staggered):
                    for _ in range(unroll):
                        body()
            # final iteration's deferred tail drain
            drain_tail()
    nc.finalize()
    return nc


def prepare_inputs(x, symbol_ids, image_ids, W1, b1, W2, b2, W3, b3, slope,
                   intercept):
    """Global stable sort by symbol, equal per-(core,symbol) dealing;
    run-boundary tables kept host-side. Returns (in_maps, metas)."""
    global _LAST_NGS
    x = np.ascontiguousarray(np.asarray(x, dtype=np.float32))
    sym = np.asarray(symbol_ids, dtype=np.int32)
    img = np.asarray(image_ids, dtype=np.int32)
    W1 = np.ascontiguousarray(np.asarray(W1, np.float32))
    W2 = np.ascontiguousarray(np.asarray(W2, np.float32))
    W3 = np.asarray(W3, np.float32)
    b1 = np.ascontiguousarray(np.asarray(b1, np.float32))
    b2 = np.ascontiguousarray(np.asarray(b2, np.float32))
    b3 = np.asarray(b3, np.float32)
    slope = np.asarray(slope, np.float32)
    intercept = np.asarray(intercept, np.float32)

    W3c = (W3 * slope[:, None]).astype(np.float32)
    cvec = (slope * b3 + intercept).astype(np.float32).reshape(1, S)

    cst = np.zeros((128, CB), ml_dtypes.bfloat16)
    for s in range(S):
        cst[:, _W1_OFF + 128 * s : _W1_OFF + 128 * (s + 1)] = W1[s]
        cst[:, _W2_OFF + 128 * s : _W2_OFF + 128 * (s + 1)] = W2[s]
        cst[:, _W3_OFF + s] = W3c[s]
        cst[:, _B1_OFF + s] = b1[s]
        cst[:, _B2_OFF + s] = b2[s]

    order = np.argsort(sym, kind="stable").astype(np.int64)
    counts = np.bincount(sym, minlength=S)
    starts = np.concatenate([[0], np.cumsum(counts)])
    gs = [(int(counts[s]) + NCORES - 1) // NCORES for s in range(S)]
    ngs = tuple((g + 127) // 128 * 128 for g in gs)
    _LAST_NGS = ngs
    pl = plan(ngs)
    NS, symbase = pl["NS"], pl["symbase"]

    in_maps, metas = [], []
    for k in range(NCORES):
        xs = np.zeros((NS, D), ml_dtypes.bfloat16)
        bnd = np.zeros(S * (B + 1), np.int64)
        cnts = np.zeros((S, B), np.int64)
        for s in range(S):
            lo = starts[s] + k * gs[s]
            hi = min(starts[s] + (k + 1) * gs[s], starts[s + 1])
            gidx = order[lo:hi]
            cnt = hi - lo
            base = symbase[s]
            xs[base : base + cnt] = x[gidx]
            gimg = img[gidx]
            ends = np.searchsorted(gimg, np.arange(B), "right")
            bnd[s * (B + 1) : s * (B + 1) + B] = base + ends - 1
            bnd[s * (B + 1) + B] = base + ngs[s] - 1
            cnts[s] = np.diff(np.concatenate([[0], ends]))
        xst = np.ascontiguousarray(xs.T)  # [D, NS] bf16
        in_maps.append(dict(xst=xst, cst=cst))
        metas.append((bnd, cnts, cvec))
    return in_maps, (metas, pl)


def finish_output(results, metas):
    """Per-image energies from device per-atom energies: host prefix sums +
    O(B) boundary diffs."""
    metas, pl = metas
    KC = pl["KC"]
    out = np.zeros(B, np.float32)
    for k in range(NCORES):
        bnd, cnts, cvec = metas[k]
        e2d = np.asarray(results[k]["e"], np.float64)  # [128, KC]
        gp = np.cumsum(e2d.T.reshape(-1))
        q = bnd
        gpv = np.where(q >= 0, gp[np.maximum(q, 0)], 0.0)
        t = np.concatenate([[0.0], gpv])
        rs = (t[1:] - t[:-1]).reshape(S, B + 1)[:, :B]
        rs = rs + cvec.reshape(S, 1) * cnts  # per-symbol affine constants
        out += rs.sum(axis=0).astype(np.float32)
    return out


_NC_CACHE = {}


def kernel(**inputs):
    in_maps, metas = prepare_inputs(**inputs)
    ngs = metas[1]["ngs"]
    if ngs not in _NC_CACHE:
        _NC_CACHE[ngs] = build_nc(ngs)
    res = run_bass_kernel_spmd(_NC_CACHE[ngs], in_maps, list(range(NCORES)))
    return finish_output(res.results, metas)
